# revision 1
# baseline (speedup 1.0000x reference)
"""AttentionPool segment-softmax-pool kernel for 8 Trainium2 NeuronCores.

Math (reference): h = x @ W.T + b, reshaped [N, 4 heads, 64];
score = h . att_w + att_b per head; leaky_relu(0.2); softmax over rows of
the same class y (1000 classes); pooled[c] = sum_n softmax_w * h.

Implementation notes:
- softmax is shift-invariant and scores here are O(1), so the segment-max
  pass is dropped: e = exp(lrelu(score)), pooled = (seg_sum e*h)/(seg_sum e).
- lin_b folds out of the hot path entirely: attention weights sum to 1 per
  (class, head), so pooled = (seg_sum e*(x@W.T))/(seg_sum e) + b.
- score = x . v_h + c_h with v_h = W_h.T @ att_w, c_h = att_w . b_h + att_b
  (weight folding on host).
- per 128-row tile, segment-sum is a one-hot matmul: a fp16 one-hot
  [128 rows, 1024 classes] is built on DVE (iota==y), and 8 class-chunk
  matmuls accumulate z = [e*h | e] (fp16, [4,65] per-head layout) into
  persistent PSUM accumulators across all tiles.
- PSUM bank map (8 banks x 512 f32): banks 0-6 = class chunks 0-6
  ([128, 260] each); chunk 7 is split into the spare space of banks 4-6
  (two N=128 matmuls + one N=4 matmul); bank 3 spare holds the score
  block; bank 7 holds the per-tile linear output h [128, 256].
- data-parallel over rows: each core gets N/8 rows; per-class partial
  sums [1024, 260] are returned per core and combined on host.
"""
import numpy as np

N_TOTAL = 500000
IN_CH = 128
OUT_CH = 64
NHEAD = 4
NUM_CLASSES = 1000
NEG_SLOPE = 0.2
NCORES = 8
ROWS_PER_CORE = N_TOTAL // NCORES          # 62500
TILES_PER_BLOCK = 8
ROWS_PER_BLOCK = 128 * TILES_PER_BLOCK     # 1024
NBLK = -(-ROWS_PER_CORE // ROWS_PER_BLOCK)  # 62
ROWS_PAD = NBLK * ROWS_PER_BLOCK           # 63488
NTILES = NBLK * TILES_PER_BLOCK            # 496
DUMP_CLASS = 1012                          # in chunk 7, >= NUM_CLASSES

_prog_cache = {}


def _build(nblk):
    try:
        from concourse.compiler_utils import (get_compiler_flags,
                                              set_compiler_flags)
        set_compiler_flags([
            s.replace("--enable-ldw-opt=false", "--enable-ldw-opt=true")
            for s in get_compiler_flags()])
    except Exception:
        pass
    import concourse.bacc as bacc
    import concourse.mybir as mybir
    from concourse import tile

    f32 = mybir.dt.float32
    fp16 = mybir.dt.float16
    fp8 = mybir.dt.float8e4
    i16 = mybir.dt.int16
    ntiles = nblk * TILES_PER_BLOCK
    nrows = nblk * ROWS_PER_BLOCK

    nc = bacc.Bacc(None, target_bir_lowering=False)

    xt_d = nc.dram_tensor("xt", [128, nrows], fp16, kind="ExternalInput")
    wvh_d = nc.dram_tensor("wvh", [128, 256], fp16, kind="ExternalInput")
    wvv_d = nc.dram_tensor("wvv", [128, 4], fp16, kind="ExternalInput")
    cvec_d = nc.dram_tensor("cvec", [128, 32], fp16, kind="ExternalInput")
    iota_d = nc.dram_tensor("iota", [128, 1024], i16, kind="ExternalInput")
    ycol_d = nc.dram_tensor("ycol", [128, ntiles], f32, kind="ExternalInput")
    part_d = nc.dram_tensor("part", [1024, 260], f32, kind="ExternalOutput")

    ps = nc.alloc_psum_tensor("ps", [128, 4096], f32).ap()
    # bank j = ps[:, 512*j : 512*(j+1)]
    accum = [ps[:, 512 * j: 512 * j + 260] for j in range(7)]
    ch7e = ps[:, 512 * 4 + 264: 512 * 4 + 268]             # [128, 4]
    ch7a = ps[:, 512 * 5 + 264: 512 * 5 + 392]             # [128, 128]
    ch7b = ps[:, 512 * 6 + 264: 512 * 6 + 392]             # [128, 128]
    h_ps = ps[:, 512 * 7: 512 * 7 + 256]                   # [128, 256]
    # bank 7 spare: h's start=True re-poisons the bank every tile, so the
    # next block's score matmuls get overwrite (not accumulate) semantics.
    score_blk = ps[:, 512 * 7 + 256: 512 * 7 + 288]        # [128, 32]

    iota_s = nc.alloc_sbuf_tensor("iota_s", [128, 1024], i16).ap()
    ycol_s = nc.alloc_sbuf_tensor("ycol_s", [128, ntiles], f32).ap()
    wvh_s = nc.alloc_sbuf_tensor("wvh_s", [128, 256], fp16).ap()
    wvv_s = nc.alloc_sbuf_tensor("wvv_s", [128, 4], fp16).ap()
    cvec_s = nc.alloc_sbuf_tensor("cvec_s", [128, 32], fp16).ap()
    stage = nc.alloc_sbuf_tensor("stage", [128, 7, 260], f32).ap()
    stage7 = nc.alloc_sbuf_tensor("stage7", [128, 260], f32).ap()

    eq = mybir.AluOpType.is_equal
    mul = mybir.AluOpType.mult
    add = mybir.AluOpType.add
    mx = mybir.AluOpType.max
    AF = mybir.ActivationFunctionType

    with tile.TileContext(nc) as tc:
        with (
            tc.tile_pool(name="io", bufs=3) as iop,
            tc.tile_pool(name="oh", bufs=4) as ohp,
            tc.tile_pool(name="zp", bufs=3) as zp,
            tc.tile_pool(name="sp", bufs=3) as sp,
        ):
            nc.sync.dma_start(iota_s, iota_d[:])
            nc.sync.dma_start(ycol_s, ycol_d[:])
            nc.sync.dma_start(wvh_s, wvh_d[:])
            nc.sync.dma_start(wvv_s, wvv_d[:])
            nc.sync.dma_start(cvec_s, cvec_d[:])

            # Software pipeline with a one-tile skew: while the PE streams
            # tile t-1's chunk matmuls, DVE/ACT build tile t's one-hot and
            # scaled z. Block b+1's scores/e are prepared two tiles before
            # the boundary so they never sit on the critical path.
            ntiles_ = ntiles

            def chunk_mms(t, oh, z, i, js):
                first = (t == 0)
                last = (t == ntiles_ - 1)
                zi = z[:, i].rearrange("p a b -> p (a b)")
                oh7 = oh[:, 896:1024]
                for j in js:
                    if j < 7:
                        nc.tensor.matmul(
                            accum[j], oh[:, 128 * j: 128 * (j + 1)], zi,
                            start=first, stop=last, skip_group_check=True)
                    elif j == 7:
                        # chunk-7 accumulators live in bank 4-6 spares:
                        # never start=True — they inherit the banks' t==0
                        # pending-zero from accum4-6 (emitted first).
                        nc.tensor.matmul(ch7a, oh7, z[:, i, 0:2, 0:64],
                                         start=False, stop=last,
                                         skip_group_check=True)
                    elif j == 8:
                        nc.tensor.matmul(ch7b, oh7, z[:, i, 2:4, 0:64],
                                         start=False, stop=last,
                                         skip_group_check=True)
                    else:
                        nc.tensor.matmul(ch7e, oh7, z[:, i, :, 64],
                                         start=False, stop=last,
                                         skip_group_check=True)

            def mk_oh(t):
                oh = ohp.tile([128, 1024], fp16)
                nc.vector.tensor_scalar(
                    oh[:], iota_s, ycol_s[:, t: t + 1], None, eq)
                return oh

            def dma_xt(b):
                xt = iop.tile([128, ROWS_PER_BLOCK], fp16)
                nc.sync.dma_start(
                    xt[:],
                    xt_d[:, b * ROWS_PER_BLOCK:(b + 1) * ROWS_PER_BLOCK])
                return xt

            def prep_block(b, xt, is_first):
                for k in range(TILES_PER_BLOCK):
                    nc.tensor.matmul(
                        score_blk[:, 4 * k: 4 * k + 4],
                        xt[:, 128 * k: 128 * (k + 1)], wvv_s,
                        start=(is_first and k == 0), stop=True,
                        skip_group_check=True)
                sc2 = sp.tile([128, 32], fp16)
                nc.vector.tensor_tensor(sc2[:], score_blk, cvec_s, add)
                sc3 = sp.tile([128, 32], fp16)
                nc.vector.scalar_tensor_tensor(
                    sc3[:], sc2[:], NEG_SLOPE, sc2[:], mul, mx)
                e_sb = sp.tile([128, 32], f32)
                nc.scalar.activation(e_sb[:], sc3[:], AF.Exp)
                z = zp.tile([128, TILES_PER_BLOCK, 4, 65], fp16)
                nc.scalar.activation(
                    z[:, :, :, 64],
                    sc3[:].rearrange("p (a b) -> p a b", a=8), AF.Exp)
                return z, e_sb

            prev = None
            oh_next = None
            xt_cur = xt_next = None
            z_cur = e_cur = z_next = e_next = None
            for t in range(ntiles):
                b, i = divmod(t, TILES_PER_BLOCK)
                if t == 0:
                    xt_cur = dma_xt(0)
                    xt_next = dma_xt(1) if nblk > 1 else None
                    z_cur, e_cur = prep_block(0, xt_cur, True)
                    oh_next = mk_oh(0)
                elif i == 0:
                    xt_cur, z_cur, e_cur = xt_next, z_next, e_next
                    xt_next = dma_xt(b + 1) if b + 1 < nblk else None
                nc.tensor.matmul(
                    h_ps, xt_cur[:, 128 * i: 128 * (i + 1)], wvh_s,
                    start=True, stop=True, skip_group_check=True)
                nc.vector.tensor_tensor(
                    z_cur[:, i, :, 0:64],
                    h_ps.rearrange("p (a b) -> p a b", a=4),
                    e_cur[:, 4 * i: 4 * i + 4].broadcast_to([128, 4, 64]),
                    mul)
                oh_cur = oh_next
                oh_next = mk_oh(t + 1) if t + 1 < ntiles else None
                if i == 6 and b + 1 < nblk:
                    if prev is not None:
                        chunk_mms(*prev, range(0, 7))
                    z_next, e_next = prep_block(b + 1, xt_next, False)
                    if prev is not None:
                        chunk_mms(*prev, range(7, 10))
                else:
                    if prev is not None:
                        chunk_mms(*prev, range(0, 10))
                prev = (t, oh_cur, z_cur, i)
            chunk_mms(*prev, range(0, 10))

            for j in range(7):
                nc.vector.tensor_copy(stage[:, j], accum[j])
            nc.vector.tensor_copy(
                stage7[:, 0:128], ch7a)
            nc.vector.tensor_copy(
                stage7[:, 128:256], ch7b)
            nc.vector.tensor_copy(stage7[:, 256:260], ch7e)
            nc.sync.dma_start(
                part_d[0:896].rearrange("(j r) d -> r j d", r=128), stage)
            nc.sync.dma_start(part_d[896:1024], stage7)

    nc.compile()
    return nc


def _get_prog(nblk):
    if nblk not in _prog_cache:
        _prog_cache[nblk] = _build(nblk)
    return _prog_cache[nblk]


def _host_prep(x, y, lin_w, lin_b, att_w, att_b, nblk=NBLK):
    """Build per-core input maps. x [R,128] f32, y [R] int32 (one shard)."""
    nrows = nblk * ROWS_PER_BLOCK
    ntiles = nblk * TILES_PER_BLOCK
    r = x.shape[0]
    xt = np.zeros((128, nrows), dtype=np.float16)
    xt[:, :r] = np.ascontiguousarray(x.T).astype(np.float16)
    ypad = np.full(nrows, DUMP_CLASS, dtype=np.int32)
    ypad[:r] = y
    ycol = np.ascontiguousarray(
        ypad.reshape(ntiles, 128).T).astype(np.float32)
    return {"xt": xt, "ycol": ycol}


def _host_weights(lin_w, lin_b, att_w, att_b):
    # wvh col layout [head, 64]: wvh[k, h*64+j] = lin_w[h*64+j, k]
    wvh = np.ascontiguousarray(lin_w.T).astype(np.float16)        # [128, 256]
    w3 = lin_w.reshape(NHEAD, OUT_CH, IN_CH).astype(np.float64)
    v = np.einsum("hjk,j->kh", w3, att_w[0].astype(np.float64))   # [128, 4]
    wvv = v.astype(np.float16)
    c = (lin_b.reshape(NHEAD, OUT_CH).astype(np.float64)
         @ att_w[0].astype(np.float64) + float(att_b[0]))          # [4]
    cvec = np.tile(np.tile(c.astype(np.float16), 8), (128, 1))  # [128, 32]
    iota = np.tile(np.arange(1024, dtype=np.int16), (128, 1))
    return {"wvh": wvh, "wvv": wvv, "cvec": cvec, "iota": iota}


def kernel(context_h_input, context_y, num_classes, lin_w, lin_b, att_w,
           att_b):
    from concourse.bass_utils import run_bass_kernel_spmd

    x = np.asarray(context_h_input, dtype=np.float32)
    y = np.asarray(context_y, dtype=np.int32)
    lin_w = np.asarray(lin_w, dtype=np.float32)
    lin_b = np.asarray(lin_b, dtype=np.float32)
    att_w = np.asarray(att_w, dtype=np.float32)
    att_b = np.asarray(att_b, dtype=np.float32)
    n = x.shape[0]
    assert int(num_classes) == NUM_CLASSES and n == N_TOTAL

    nc = _get_prog(NBLK)
    wmap = _host_weights(lin_w, lin_b, att_w, att_b)
    in_maps = []
    for i in range(NCORES):
        lo, hi = i * ROWS_PER_CORE, (i + 1) * ROWS_PER_CORE
        m = _host_prep(x[lo:hi], y[lo:hi], lin_w, lin_b, att_w, att_b)
        m.update(wmap)
        in_maps.append(m)

    res = run_bass_kernel_spmd(nc, in_maps, list(range(NCORES)))
    p = np.zeros((1024, 260), dtype=np.float64)
    for r in res.results:
        p += r["part"].astype(np.float64)

    pooled = np.empty((NUM_CLASSES, NHEAD, OUT_CH), dtype=np.float64)
    denom = np.empty((NUM_CLASSES, NHEAD), dtype=np.float64)
    pc = p[:896].reshape(896, NHEAD, 65)
    pooled[:896] = pc[:, :, 0:64]
    denom[:896] = pc[:, :, 64]
    p7 = p[896:896 + 104]
    pooled[896:] = p7[:, 0:256].reshape(104, NHEAD, OUT_CH)
    denom[896:] = p7[:, 256:260]
    out = pooled / denom[:, :, None] + lin_b.astype(np.float64).reshape(
        NHEAD, OUT_CH)[None]
    return out.reshape(NUM_CLASSES, NHEAD * OUT_CH).astype(np.float32)



# revision 2
# speedup vs baseline: 1.3558x; 1.3558x over previous
"""AttentionPool segment-softmax-pool kernel for 8 Trainium2 NeuronCores.

Math (reference): h = x @ W.T + b, reshaped [N, 4 heads, 64];
score = h . att_w + att_b per head; leaky_relu(0.2); softmax over rows of
the same class y (1000 classes); pooled[c] = sum_n softmax_w * h.

Implementation notes:
- softmax is shift-invariant and scores here are O(1), so the segment-max
  pass is dropped: e = exp(lrelu(score)), pooled = (seg_sum e*h)/(seg_sum e).
- lin_b folds out of the hot path entirely: attention weights sum to 1 per
  (class, head), so pooled = (seg_sum e*(x@W.T))/(seg_sum e) + b.
- score = x . v_h + c_h with v_h = W_h.T @ att_w, c_h = att_w . b_h + att_b
  (weight folding on host).
- class-sharded data parallelism: rows are argsorted by y on the host and
  core k gets exactly classes [125k, 125(k+1)).  Each core's class window
  fits 128 one-hot slots, so the per-tile segment-sum is a SINGLE matmul:
  a fp16 one-hot [128 rows, 128 slots] built on DVE (iota==slot) times
  z = [e*h | e] (fp16, [4,65] per-head layout), accumulated into one
  persistent PSUM bank [128, 260] across all tiles.  Class windows are
  disjoint across cores, so the host just concatenates (no all-reduce).
- PSUM bank map: bank 0 = window accumulator [128, 260]; bank 7 holds the
  per-tile linear output h [128, 256] plus the score block in its spare
  (h's start=True re-poisons the bank so scores get overwrite semantics).
- slot 125 is the dump slot for rows padding the shard to a block multiple.
"""
import numpy as np

N_TOTAL = 500000
IN_CH = 128
OUT_CH = 64
NHEAD = 4
NUM_CLASSES = 1000
NEG_SLOPE = 0.2
NCORES = 8
CLS_PER_CORE = NUM_CLASSES // NCORES       # 125
ROWS_PER_CORE = N_TOTAL // NCORES          # 62500 (average; shards vary)
TILES_PER_BLOCK = 8
ROWS_PER_BLOCK = 128 * TILES_PER_BLOCK     # 1024
NBLK = -(-ROWS_PER_CORE // ROWS_PER_BLOCK)  # 62
DUMP_SLOT = 125                            # >= CLS_PER_CORE, < 128

_prog_cache = {}


def _build(nblk):
    try:
        from concourse.compiler_utils import (get_compiler_flags,
                                              set_compiler_flags)
        set_compiler_flags([
            s.replace("--enable-ldw-opt=false", "--enable-ldw-opt=true")
            for s in get_compiler_flags()])
    except Exception:
        pass
    import concourse.bacc as bacc
    import concourse.mybir as mybir
    from concourse import tile

    f32 = mybir.dt.float32
    fp16 = mybir.dt.float16
    i16 = mybir.dt.int16
    ntiles = nblk * TILES_PER_BLOCK
    nrows = nblk * ROWS_PER_BLOCK

    nc = bacc.Bacc(None, target_bir_lowering=False)

    xt_d = nc.dram_tensor("xt", [128, nrows], fp16, kind="ExternalInput")
    wvh_d = nc.dram_tensor("wvh", [128, 256], fp16, kind="ExternalInput")
    wvv_d = nc.dram_tensor("wvv", [128, 4], fp16, kind="ExternalInput")
    cvec_d = nc.dram_tensor("cvec", [128, 32], fp16, kind="ExternalInput")
    iota_d = nc.dram_tensor("iota", [128, 128], i16, kind="ExternalInput")
    ycol_d = nc.dram_tensor("ycol", [128, ntiles], f32, kind="ExternalInput")
    part_d = nc.dram_tensor("part", [128, 260], f32, kind="ExternalOutput")

    ps = nc.alloc_psum_tensor("ps", [128, 4096], f32).ap()
    # bank j = ps[:, 512*j : 512*(j+1)]
    accum = ps[:, 0:260]                                   # bank 0
    h_ps = ps[:, 512 * 7: 512 * 7 + 256]                   # [128, 256]
    # bank 7 spare: h's start=True re-poisons the bank every tile, so the
    # next block's score matmuls get overwrite (not accumulate) semantics.
    score_blk = ps[:, 512 * 7 + 256: 512 * 7 + 288]        # [128, 32]

    iota_s = nc.alloc_sbuf_tensor("iota_s", [128, 128], i16).ap()
    ycol_s = nc.alloc_sbuf_tensor("ycol_s", [128, ntiles], f32).ap()
    wvh_s = nc.alloc_sbuf_tensor("wvh_s", [128, 256], fp16).ap()
    wvv_s = nc.alloc_sbuf_tensor("wvv_s", [128, 4], fp16).ap()
    cvec_s = nc.alloc_sbuf_tensor("cvec_s", [128, 32], fp16).ap()
    stage = nc.alloc_sbuf_tensor("stage", [128, 260], f32).ap()

    eq = mybir.AluOpType.is_equal
    mul = mybir.AluOpType.mult
    add = mybir.AluOpType.add
    mx = mybir.AluOpType.max
    AF = mybir.ActivationFunctionType

    with tile.TileContext(nc) as tc:
        with (
            tc.tile_pool(name="io", bufs=3) as iop,
            tc.tile_pool(name="oh", bufs=4) as ohp,
            tc.tile_pool(name="zp", bufs=3) as zp,
            tc.tile_pool(name="sp", bufs=3) as sp,
        ):
            nc.sync.dma_start(iota_s, iota_d[:])
            nc.sync.dma_start(ycol_s, ycol_d[:])
            nc.sync.dma_start(wvh_s, wvh_d[:])
            nc.sync.dma_start(wvv_s, wvv_d[:])
            nc.sync.dma_start(cvec_s, cvec_d[:])

            # Software pipeline with a one-tile skew: while the PE streams
            # tile t-1's segment matmul, DVE/ACT build tile t's one-hot and
            # scaled z. Block b+1's scores/e are prepared two tiles before
            # the boundary so they never sit on the critical path.
            ntiles_ = ntiles

            def seg_mm(t, oh, z, i):
                zi = z[:, i].rearrange("p a b -> p (a b)")
                nc.tensor.matmul(
                    accum, oh[:], zi,
                    start=(t == 0), stop=(t == ntiles_ - 1),
                    skip_group_check=True)

            def mk_oh(t):
                oh = ohp.tile([128, 128], fp16)
                nc.vector.tensor_scalar(
                    oh[:], iota_s, ycol_s[:, t: t + 1], None, eq)
                return oh

            def dma_xt(b):
                xt = iop.tile([128, ROWS_PER_BLOCK], fp16)
                nc.sync.dma_start(
                    xt[:],
                    xt_d[:, b * ROWS_PER_BLOCK:(b + 1) * ROWS_PER_BLOCK])
                return xt

            def prep_block(b, xt, is_first):
                for k in range(TILES_PER_BLOCK):
                    nc.tensor.matmul(
                        score_blk[:, 4 * k: 4 * k + 4],
                        xt[:, 128 * k: 128 * (k + 1)], wvv_s,
                        start=(is_first and k == 0), stop=True,
                        skip_group_check=True)
                sc2 = sp.tile([128, 32], fp16)
                nc.vector.tensor_tensor(sc2[:], score_blk, cvec_s, add)
                sc3 = sp.tile([128, 32], fp16)
                nc.vector.scalar_tensor_tensor(
                    sc3[:], sc2[:], NEG_SLOPE, sc2[:], mul, mx)
                e_sb = sp.tile([128, 32], f32)
                nc.scalar.activation(e_sb[:], sc3[:], AF.Exp)
                z = zp.tile([128, TILES_PER_BLOCK, 4, 65], fp16)
                nc.scalar.activation(
                    z[:, :, :, 64],
                    sc3[:].rearrange("p (a b) -> p a b", a=8), AF.Exp)
                return z, e_sb

            prev = None
            oh_next = None
            xt_cur = xt_next = None
            z_cur = e_cur = z_next = e_next = None
            for t in range(ntiles):
                b, i = divmod(t, TILES_PER_BLOCK)
                if t == 0:
                    xt_cur = dma_xt(0)
                    xt_next = dma_xt(1) if nblk > 1 else None
                    z_cur, e_cur = prep_block(0, xt_cur, True)
                    oh_next = mk_oh(0)
                elif i == 0:
                    xt_cur, z_cur, e_cur = xt_next, z_next, e_next
                    xt_next = dma_xt(b + 1) if b + 1 < nblk else None
                nc.tensor.matmul(
                    h_ps, xt_cur[:, 128 * i: 128 * (i + 1)], wvh_s,
                    start=True, stop=True, skip_group_check=True)
                nc.vector.tensor_tensor(
                    z_cur[:, i, :, 0:64],
                    h_ps.rearrange("p (a b) -> p a b", a=4),
                    e_cur[:, 4 * i: 4 * i + 4].broadcast_to([128, 4, 64]),
                    mul)
                oh_cur = oh_next
                oh_next = mk_oh(t + 1) if t + 1 < ntiles else None
                if prev is not None:
                    seg_mm(*prev)
                if i == 6 and b + 1 < nblk:
                    z_next, e_next = prep_block(b + 1, xt_next, False)
                prev = (t, oh_cur, z_cur, i)
            seg_mm(*prev)

            nc.vector.tensor_copy(stage[:], accum)
            nc.sync.dma_start(part_d[:], stage)

    nc.compile()
    return nc


def _get_prog(nblk):
    if nblk not in _prog_cache:
        _prog_cache[nblk] = _build(nblk)
    return _prog_cache[nblk]


def _host_weights(lin_w, lin_b, att_w, att_b):
    # wvh col layout [head, 64]: wvh[k, h*64+j] = lin_w[h*64+j, k]
    wvh = np.ascontiguousarray(lin_w.T).astype(np.float16)        # [128, 256]
    w3 = lin_w.reshape(NHEAD, OUT_CH, IN_CH).astype(np.float64)
    v = np.einsum("hjk,j->kh", w3, att_w[0].astype(np.float64))   # [128, 4]
    wvv = v.astype(np.float16)
    c = (lin_b.reshape(NHEAD, OUT_CH).astype(np.float64)
         @ att_w[0].astype(np.float64) + float(att_b[0]))          # [4]
    cvec = np.tile(np.tile(c.astype(np.float16), 8), (128, 1))  # [128, 32]
    iota = np.tile(np.arange(128, dtype=np.int16), (128, 1))
    return {"wvh": wvh, "wvv": wvv, "cvec": cvec, "iota": iota}


def _prepare(x, y, lin_w, lin_b, att_w, att_b):
    """Sort rows by class, shard into disjoint 125-class windows, build
    per-core input maps.  Returns (nblk, in_maps)."""
    order = np.argsort(y, kind="stable")
    ys = y[order]
    bounds = np.searchsorted(
        ys, np.arange(0, NUM_CLASSES + 1, CLS_PER_CORE)).astype(np.int64)
    max_rows = int(np.diff(bounds).max())
    nblk = max(NBLK, -(-max_rows // ROWS_PER_BLOCK))
    nrows = nblk * ROWS_PER_BLOCK
    ntiles = nblk * TILES_PER_BLOCK
    wmap = _host_weights(lin_w, lin_b, att_w, att_b)
    in_maps = []
    for k in range(NCORES):
        lo, hi = int(bounds[k]), int(bounds[k + 1])
        r = hi - lo
        xt = np.zeros((128, nrows), dtype=np.float16)
        xt[:, :r] = x[order[lo:hi]].T.astype(np.float16)
        spad = np.full(nrows, DUMP_SLOT, dtype=np.int32)
        spad[:r] = ys[lo:hi] - k * CLS_PER_CORE
        ycol = np.ascontiguousarray(
            spad.reshape(ntiles, 128).T).astype(np.float32)
        m = {"xt": xt, "ycol": ycol}
        m.update(wmap)
        in_maps.append(m)
    return nblk, in_maps


def kernel(context_h_input, context_y, num_classes, lin_w, lin_b, att_w,
           att_b):
    from concourse.bass_utils import run_bass_kernel_spmd

    x = np.asarray(context_h_input, dtype=np.float32)
    y = np.asarray(context_y, dtype=np.int32)
    lin_w = np.asarray(lin_w, dtype=np.float32)
    lin_b = np.asarray(lin_b, dtype=np.float32)
    att_w = np.asarray(att_w, dtype=np.float32)
    att_b = np.asarray(att_b, dtype=np.float32)
    n = x.shape[0]
    assert int(num_classes) == NUM_CLASSES and n == N_TOTAL

    nblk, in_maps = _prepare(x, y, lin_w, lin_b, att_w, att_b)
    nc = _get_prog(nblk)
    res = run_bass_kernel_spmd(nc, in_maps, list(range(NCORES)))

    pooled = np.empty((NUM_CLASSES, NHEAD, OUT_CH), dtype=np.float64)
    denom = np.empty((NUM_CLASSES, NHEAD), dtype=np.float64)
    for k, r in enumerate(res.results):
        rc = r["part"][:CLS_PER_CORE].astype(np.float64).reshape(
            CLS_PER_CORE, NHEAD, 65)
        pooled[k * CLS_PER_CORE:(k + 1) * CLS_PER_CORE] = rc[:, :, 0:64]
        denom[k * CLS_PER_CORE:(k + 1) * CLS_PER_CORE] = rc[:, :, 64]
    out = pooled / denom[:, :, None] + lin_b.astype(np.float64).reshape(
        NHEAD, OUT_CH)[None]
    return out.reshape(NUM_CLASSES, NHEAD * OUT_CH).astype(np.float32)


# revision 3
# speedup vs baseline: 1.7167x; 1.2662x over previous
"""AttentionPool segment-softmax-pool kernel for 8 Trainium2 NeuronCores.

Math (reference): h = x @ W.T + b, reshaped [N, 4 heads, 64];
score = h . att_w + att_b per head; leaky_relu(0.2); softmax over rows of
the same class y (1000 classes); pooled[c] = sum_n softmax_w * h.

Implementation notes:
- softmax is shift-invariant and scores here are O(1), so the segment-max
  pass is dropped: e = exp(lrelu(score)), pooled = (seg_sum e*h)/(seg_sum e).
- lin_b folds out of the hot path entirely: attention weights sum to 1 per
  (class, head), so pooled = (seg_sum e*(x@W.T))/(seg_sum e) + b.
- score = x . v_h + c_h with v_h = W_h.T @ att_w, c_h = att_w . b_h + att_b
  (weight folding on host).
- class-sharded data parallelism: rows are argsorted by y on the host and
  core k gets exactly classes [125k, 125(k+1)).  Each core's class window
  fits 128 one-hot slots, so the per-tile segment-sum is a SINGLE matmul:
  the fp16 one-hot [128 rows, 128 slots] is precomputed on the host and
  DMA'd (hides under PE), times z = [e*h | e] (fp16, [4,65] per-head
  layout).  Class windows are disjoint across cores: host concatenates.
- PSUM RMW turnaround: back-to-back accumulation into one bank stalls the
  PE ~4x, so tiles alternate between two accumulator banks (merged on
  host); the per-tile h output likewise ping-pongs banks 6/7 so the PE
  never waits for DVE/ACT readers of the previous tile.
- z build: ACT copies h (PSUM f32) to SBUF fp16, DVE multiplies by e in
  fp16 (PSUM-sourced tensor_tensor would run at 1x).
- slot 125 is the dump slot for rows padding the shard to a block multiple.
"""
import numpy as np

N_TOTAL = 500000
IN_CH = 128
OUT_CH = 64
NHEAD = 4
NUM_CLASSES = 1000
NEG_SLOPE = 0.2
NCORES = 8
CLS_PER_CORE = NUM_CLASSES // NCORES       # 125
ROWS_PER_CORE = N_TOTAL // NCORES          # 62500 (average; shards vary)
TILES_PER_BLOCK = 8
ROWS_PER_BLOCK = 128 * TILES_PER_BLOCK     # 1024
NBLK = -(-ROWS_PER_CORE // ROWS_PER_BLOCK)  # 62
DUMP_SLOT = 125                            # >= CLS_PER_CORE, < 128

_prog_cache = {}


def _build(nblk):
    try:
        from concourse.compiler_utils import (get_compiler_flags,
                                              set_compiler_flags)
        set_compiler_flags([
            s.replace("--enable-ldw-opt=false", "--enable-ldw-opt=true")
            for s in get_compiler_flags()])
    except Exception:
        pass
    import concourse.bacc as bacc
    import concourse.mybir as mybir
    from concourse import tile

    f32 = mybir.dt.float32
    fp16 = mybir.dt.float16
    ntiles = nblk * TILES_PER_BLOCK
    nrows = nblk * ROWS_PER_BLOCK

    nc = bacc.Bacc(None, target_bir_lowering=False)

    xt_d = nc.dram_tensor("xt", [128, nrows], fp16, kind="ExternalInput")
    oh_d = nc.dram_tensor("oh", [nrows, 128], fp16, kind="ExternalInput")
    wvh_d = nc.dram_tensor("wvh", [128, 256], fp16, kind="ExternalInput")
    wvv_d = nc.dram_tensor("wvv", [128, 4], fp16, kind="ExternalInput")
    cvec_d = nc.dram_tensor("cvec", [128, 32], fp16, kind="ExternalInput")
    part_d = nc.dram_tensor("part", [128, 520], f32, kind="ExternalOutput")

    ps = nc.alloc_psum_tensor("ps", [128, 4096], f32).ap()
    # bank j = ps[:, 512*j : 512*(j+1)]
    acc = [ps[:, 0:260], ps[:, 512: 512 + 260]]            # banks 0/1
    h_ps = [ps[:, 512 * 7: 512 * 7 + 256],                 # even tiles
            ps[:, 512 * 6: 512 * 6 + 256]]                 # odd tiles
    # bank 7 spare: h's start=True re-poisons the bank every other tile, so
    # the next block's score matmuls get overwrite (not accumulate)
    # semantics.
    score_blk = ps[:, 512 * 7 + 256: 512 * 7 + 288]        # [128, 32]

    wvh_s = nc.alloc_sbuf_tensor("wvh_s", [128, 256], fp16).ap()
    wvv_s = nc.alloc_sbuf_tensor("wvv_s", [128, 4], fp16).ap()
    cvec_s = nc.alloc_sbuf_tensor("cvec_s", [128, 32], fp16).ap()
    stage = nc.alloc_sbuf_tensor("stage", [128, 2, 260], f32).ap()

    mul = mybir.AluOpType.mult
    add = mybir.AluOpType.add
    mx = mybir.AluOpType.max
    AF = mybir.ActivationFunctionType

    with tile.TileContext(nc) as tc:
        with (
            tc.tile_pool(name="io", bufs=3) as iop,
            tc.tile_pool(name="ohio", bufs=3) as ohiop,
            tc.tile_pool(name="zp", bufs=3) as zp,
            tc.tile_pool(name="hp", bufs=3) as hp,
            tc.tile_pool(name="sp", bufs=3) as sp,
        ):
            nc.sync.dma_start(wvh_s, wvh_d[:])
            nc.sync.dma_start(wvv_s, wvv_d[:])
            nc.sync.dma_start(cvec_s, cvec_d[:])

            # Software pipeline with a one-tile skew: while the PE streams
            # tile t-1's segment matmul, ACT/DVE build tile t's z. Block
            # b+1's scores/e are prepared two tiles before the boundary so
            # they never sit on the critical path.
            ntiles_ = ntiles

            def seg_mm(t, ohb, z, i):
                zi = z[:, i].rearrange("p a b -> p (a b)")
                nc.tensor.matmul(
                    acc[t % 2], ohb[:, i], zi,
                    start=(t < 2), stop=(t >= ntiles_ - 2),
                    skip_group_check=True)

            def dma_blk(b):
                xt = iop.tile([128, ROWS_PER_BLOCK], fp16)
                nc.sync.dma_start(
                    xt[:],
                    xt_d[:, b * ROWS_PER_BLOCK:(b + 1) * ROWS_PER_BLOCK])
                ohb = ohiop.tile([128, TILES_PER_BLOCK, 128], fp16)
                nc.sync.dma_start(
                    ohb[:],
                    oh_d[b * ROWS_PER_BLOCK:(b + 1) * ROWS_PER_BLOCK]
                    .rearrange("(t p) s -> p t s", p=128))
                return xt, ohb

            def prep_block(b, xt, is_first):
                for k in range(TILES_PER_BLOCK):
                    nc.tensor.matmul(
                        score_blk[:, 4 * k: 4 * k + 4],
                        xt[:, 128 * k: 128 * (k + 1)], wvv_s,
                        start=(is_first and k == 0), stop=True,
                        skip_group_check=True)
                sc2 = sp.tile([128, 32], fp16)
                nc.vector.tensor_tensor(sc2[:], score_blk, cvec_s, add)
                sc3 = sp.tile([128, 32], fp16)
                nc.vector.scalar_tensor_tensor(
                    sc3[:], sc2[:], NEG_SLOPE, sc2[:], mul, mx)
                e_sb = sp.tile([128, 32], f32)
                nc.scalar.activation(e_sb[:], sc3[:], AF.Exp)
                z = zp.tile([128, TILES_PER_BLOCK, 4, 65], fp16)
                nc.scalar.activation(
                    z[:, :, :, 64],
                    sc3[:].rearrange("p (a b) -> p a b", a=8), AF.Exp)
                return z, e_sb

            prev = None
            xt_cur = xt_next = oh_cur = oh_next = None
            z_cur = e_cur = z_next = e_next = None
            for t in range(ntiles):
                b, i = divmod(t, TILES_PER_BLOCK)
                if t == 0:
                    xt_cur, oh_cur = dma_blk(0)
                    if nblk > 1:
                        xt_next, oh_next = dma_blk(1)
                    z_cur, e_cur = prep_block(0, xt_cur, True)
                elif i == 0:
                    xt_cur, oh_cur = xt_next, oh_next
                    z_cur, e_cur = z_next, e_next
                    if b + 1 < nblk:
                        xt_next, oh_next = dma_blk(b + 1)
                nc.tensor.matmul(
                    h_ps[t % 2], xt_cur[:, 128 * i: 128 * (i + 1)], wvh_s,
                    start=True, stop=True, skip_group_check=True)
                hsb = hp.tile([128, 256], fp16)
                nc.scalar.activation(hsb[:], h_ps[t % 2], AF.Copy)
                nc.vector.tensor_tensor(
                    z_cur[:, i, :, 0:64],
                    hsb[:].rearrange("p (a b) -> p a b", a=4),
                    e_cur[:, 4 * i: 4 * i + 4].broadcast_to([128, 4, 64]),
                    mul)
                if prev is not None:
                    seg_mm(*prev)
                if i == 6 and b + 1 < nblk:
                    z_next, e_next = prep_block(b + 1, xt_next, False)
                prev = (t, oh_cur, z_cur, i)
            seg_mm(*prev)

            nc.vector.tensor_copy(stage[:, 0], acc[0])
            nc.vector.tensor_copy(stage[:, 1], acc[1])
            nc.sync.dma_start(
                part_d[:], stage.rearrange("p a b -> p (a b)"))

    nc.compile()
    return nc


def _get_prog(nblk):
    if nblk not in _prog_cache:
        _prog_cache[nblk] = _build(nblk)
    return _prog_cache[nblk]


def _host_weights(lin_w, lin_b, att_w, att_b):
    # wvh col layout [head, 64]: wvh[k, h*64+j] = lin_w[h*64+j, k]
    wvh = np.ascontiguousarray(lin_w.T).astype(np.float16)        # [128, 256]
    w3 = lin_w.reshape(NHEAD, OUT_CH, IN_CH).astype(np.float64)
    v = np.einsum("hjk,j->kh", w3, att_w[0].astype(np.float64))   # [128, 4]
    wvv = v.astype(np.float16)
    c = (lin_b.reshape(NHEAD, OUT_CH).astype(np.float64)
         @ att_w[0].astype(np.float64) + float(att_b[0]))          # [4]
    cvec = np.tile(np.tile(c.astype(np.float16), 8), (128, 1))  # [128, 32]
    return {"wvh": wvh, "wvv": wvv, "cvec": cvec}


def _prepare(x, y, lin_w, lin_b, att_w, att_b):
    """Sort rows by class, shard into disjoint 125-class windows, build
    per-core input maps.  Returns (nblk, in_maps)."""
    order = np.argsort(y, kind="stable")
    ys = y[order]
    bounds = np.searchsorted(
        ys, np.arange(0, NUM_CLASSES + 1, CLS_PER_CORE)).astype(np.int64)
    max_rows = int(np.diff(bounds).max())
    nblk = max(NBLK, -(-max_rows // ROWS_PER_BLOCK))
    nrows = nblk * ROWS_PER_BLOCK
    slot_eye = np.eye(128, dtype=np.float16)
    wmap = _host_weights(lin_w, lin_b, att_w, att_b)
    in_maps = []
    for k in range(NCORES):
        lo, hi = int(bounds[k]), int(bounds[k + 1])
        r = hi - lo
        xt = np.zeros((128, nrows), dtype=np.float16)
        xt[:, :r] = x[order[lo:hi]].T.astype(np.float16)
        spad = np.full(nrows, DUMP_SLOT, dtype=np.int32)
        spad[:r] = ys[lo:hi] - k * CLS_PER_CORE
        oh = slot_eye[spad]                                # [nrows, 128]
        m = {"xt": xt, "oh": oh}
        m.update(wmap)
        in_maps.append(m)
    return nblk, in_maps


def kernel(context_h_input, context_y, num_classes, lin_w, lin_b, att_w,
           att_b):
    from concourse.bass_utils import run_bass_kernel_spmd

    x = np.asarray(context_h_input, dtype=np.float32)
    y = np.asarray(context_y, dtype=np.int32)
    lin_w = np.asarray(lin_w, dtype=np.float32)
    lin_b = np.asarray(lin_b, dtype=np.float32)
    att_w = np.asarray(att_w, dtype=np.float32)
    att_b = np.asarray(att_b, dtype=np.float32)
    n = x.shape[0]
    assert int(num_classes) == NUM_CLASSES and n == N_TOTAL

    nblk, in_maps = _prepare(x, y, lin_w, lin_b, att_w, att_b)
    nc = _get_prog(nblk)
    res = run_bass_kernel_spmd(nc, in_maps, list(range(NCORES)))

    pooled = np.empty((NUM_CLASSES, NHEAD, OUT_CH), dtype=np.float64)
    denom = np.empty((NUM_CLASSES, NHEAD), dtype=np.float64)
    for k, r in enumerate(res.results):
        p = r["part"].astype(np.float64).reshape(128, 2, 260).sum(axis=1)
        rc = p[:CLS_PER_CORE].reshape(CLS_PER_CORE, NHEAD, 65)
        pooled[k * CLS_PER_CORE:(k + 1) * CLS_PER_CORE] = rc[:, :, 0:64]
        denom[k * CLS_PER_CORE:(k + 1) * CLS_PER_CORE] = rc[:, :, 64]
    out = pooled / denom[:, :, None] + lin_b.astype(np.float64).reshape(
        NHEAD, OUT_CH)[None]
    return out.reshape(NUM_CLASSES, NHEAD * OUT_CH).astype(np.float32)


# revision 4
# speedup vs baseline: 1.7608x; 1.0257x over previous
"""AttentionPool segment-softmax-pool kernel for 8 Trainium2 NeuronCores.

Math (reference): h = x @ W.T + b, reshaped [N, 4 heads, 64];
score = h . att_w + att_b per head; leaky_relu(0.2); softmax over rows of
the same class y (1000 classes); pooled[c] = sum_n softmax_w * h.

Implementation notes:
- softmax is shift-invariant and scores here are O(1), so the segment-max
  pass is dropped: e = exp(lrelu(score)), pooled = (seg_sum e*h)/(seg_sum e).
- lin_b folds out of the hot path entirely: attention weights sum to 1 per
  (class, head), so pooled = (seg_sum e*(x@W.T))/(seg_sum e) + b.
- score = x . v_h + c_h with v_h = W_h.T @ att_w, c_h = att_w . b_h + att_b
  (weight folding on host).
- class-sharded data parallelism: rows are argsorted by y on the host and
  core k gets exactly classes [125k, 125(k+1)).  Each core's class window
  fits 128 one-hot slots, so the per-tile segment-sum is a SINGLE matmul:
  the fp16 one-hot [128 rows, 128 slots] is precomputed on the host and
  DMA'd (hides under PE), times z = [e*h | e] (fp16, [4,65] per-head
  layout).  Class windows are disjoint across cores: host concatenates.
- PSUM RMW turnaround: back-to-back accumulation into one bank stalls the
  PE ~4x, so tiles alternate between two accumulator banks (merged on
  host); the per-tile h output likewise ping-pongs banks 6/7 so the PE
  never waits for DVE/ACT readers of the previous tile.
- z build: ACT copies h (PSUM f32) to SBUF fp16, DVE multiplies by e in
  fp16 (PSUM-sourced tensor_tensor would run at 1x).
- slot 125 is the dump slot for rows padding the shard to a block multiple.
"""
import numpy as np

N_TOTAL = 500000
IN_CH = 128
OUT_CH = 64
NHEAD = 4
NUM_CLASSES = 1000
NEG_SLOPE = 0.2
NCORES = 8
CLS_PER_CORE = NUM_CLASSES // NCORES       # 125
ROWS_PER_CORE = N_TOTAL // NCORES          # 62500 (average; shards vary)
TILES_PER_BLOCK = 8
ROWS_PER_BLOCK = 128 * TILES_PER_BLOCK     # 1024
NBLK = -(-ROWS_PER_CORE // ROWS_PER_BLOCK)  # 62
DUMP_SLOT = 125                            # >= CLS_PER_CORE, < 128

_prog_cache = {}


def _build(nblk):
    try:
        from concourse.compiler_utils import (get_compiler_flags,
                                              set_compiler_flags)
        set_compiler_flags([
            s.replace("--enable-ldw-opt=false", "--enable-ldw-opt=true")
            for s in get_compiler_flags()])
    except Exception:
        pass
    import concourse.bacc as bacc
    import concourse.mybir as mybir
    from concourse import tile

    f32 = mybir.dt.float32
    fp16 = mybir.dt.float16
    ntiles = nblk * TILES_PER_BLOCK
    nrows = nblk * ROWS_PER_BLOCK

    nc = bacc.Bacc(None, target_bir_lowering=False)

    xt_d = nc.dram_tensor("xt", [128, nrows], fp16, kind="ExternalInput")
    oh_d = nc.dram_tensor("oh", [nrows, 128], fp16, kind="ExternalInput")
    wvh_d = nc.dram_tensor("wvh", [128, 256], fp16, kind="ExternalInput")
    wvv_d = nc.dram_tensor("wvv", [128, 4], fp16, kind="ExternalInput")
    cvec_d = nc.dram_tensor("cvec", [128, 32], fp16, kind="ExternalInput")
    part_d = nc.dram_tensor("part", [128, 520], f32, kind="ExternalOutput")

    ps = nc.alloc_psum_tensor("ps", [128, 4096], f32).ap()
    # bank j = ps[:, 512*j : 512*(j+1)]
    acc = [ps[:, 0:260], ps[:, 512: 512 + 260]]            # banks 0/1
    h_ps = [ps[:, 512 * 7: 512 * 7 + 256],                 # even tiles
            ps[:, 512 * 6: 512 * 6 + 256]]                 # odd tiles
    # bank 7 spare: h's start=True re-poisons the bank every other tile, so
    # the next block's score matmuls get overwrite (not accumulate)
    # semantics.
    score_blk = ps[:, 512 * 7 + 256: 512 * 7 + 288]        # [128, 32]

    wvh_s = nc.alloc_sbuf_tensor("wvh_s", [128, 256], fp16).ap()
    wvv_s = nc.alloc_sbuf_tensor("wvv_s", [128, 4], fp16).ap()
    cvec_s = nc.alloc_sbuf_tensor("cvec_s", [128, 32], fp16).ap()
    stage = nc.alloc_sbuf_tensor("stage", [128, 2, 260], f32).ap()

    mul = mybir.AluOpType.mult
    add = mybir.AluOpType.add
    mx = mybir.AluOpType.max
    AF = mybir.ActivationFunctionType

    with tile.TileContext(nc) as tc:
        with (
            tc.tile_pool(name="io", bufs=3) as iop,
            tc.tile_pool(name="ohio", bufs=3) as ohiop,
            tc.tile_pool(name="zp", bufs=3) as zp,
            tc.tile_pool(name="hp", bufs=3) as hp,
            tc.tile_pool(name="sp", bufs=3) as sp,
        ):
            nc.sync.dma_start(wvh_s, wvh_d[:])
            nc.sync.dma_start(wvv_s, wvv_d[:])
            nc.sync.dma_start(cvec_s, cvec_d[:])

            # Software pipeline with a one-tile skew: while the PE streams
            # tile t-1's segment matmul, ACT/DVE build tile t's z. Block
            # b+1's scores/e are prepared two tiles before the boundary so
            # they never sit on the critical path.
            ntiles_ = ntiles

            def seg_mm(t, ohb, z, i):
                zi = z[:, i].rearrange("p a b -> p (a b)")
                nc.tensor.matmul(
                    acc[t % 2], ohb[:, i], zi,
                    start=(t < 2), stop=(t >= ntiles_ - 2),
                    skip_group_check=True)

            def dma_blk(b):
                xt = iop.tile([128, ROWS_PER_BLOCK], fp16)
                nc.sync.dma_start(
                    xt[:],
                    xt_d[:, b * ROWS_PER_BLOCK:(b + 1) * ROWS_PER_BLOCK])
                ohb = ohiop.tile([128, TILES_PER_BLOCK, 128], fp16)
                nc.sync.dma_start(
                    ohb[:],
                    oh_d[b * ROWS_PER_BLOCK:(b + 1) * ROWS_PER_BLOCK]
                    .rearrange("(t p) s -> p t s", p=128))
                return xt, ohb

            def prep_block(b, xt, is_first):
                for k in range(TILES_PER_BLOCK):
                    nc.tensor.matmul(
                        score_blk[:, 4 * k: 4 * k + 4],
                        xt[:, 128 * k: 128 * (k + 1)], wvv_s,
                        start=(is_first and k == 0), stop=True,
                        skip_group_check=True)
                sc2 = sp.tile([128, 32], fp16)
                nc.vector.tensor_tensor(sc2[:], score_blk, cvec_s, add)
                sc3 = sp.tile([128, 32], fp16)
                nc.vector.scalar_tensor_tensor(
                    sc3[:], sc2[:], NEG_SLOPE, sc2[:], mul, mx)
                e_sb = sp.tile([128, 32], f32)
                nc.scalar.activation(e_sb[:], sc3[:], AF.Exp)
                z = zp.tile([128, TILES_PER_BLOCK, 4, 65], fp16)
                nc.scalar.activation(
                    z[:, :, :, 64],
                    sc3[:].rearrange("p (a b) -> p a b", a=8), AF.Exp)
                return z, e_sb

            prev = None
            xt_cur = xt_next = oh_cur = oh_next = None
            z_cur = e_cur = z_next = e_next = None
            for t in range(ntiles):
                b, i = divmod(t, TILES_PER_BLOCK)
                if t == 0:
                    xt_cur, oh_cur = dma_blk(0)
                    if nblk > 1:
                        xt_next, oh_next = dma_blk(1)
                    z_cur, e_cur = prep_block(0, xt_cur, True)
                elif i == 0:
                    xt_cur, oh_cur = xt_next, oh_next
                    z_cur, e_cur = z_next, e_next
                    if b + 1 < nblk:
                        xt_next, oh_next = dma_blk(b + 1)
                nc.tensor.matmul(
                    h_ps[t % 2], xt_cur[:, 128 * i: 128 * (i + 1)], wvh_s,
                    start=True, stop=True, skip_group_check=True)
                nc.vector.tensor_tensor(
                    z_cur[:, i, :, 0:64],
                    h_ps[t % 2].rearrange("p (a b) -> p a b", a=4),
                    e_cur[:, 4 * i: 4 * i + 4].broadcast_to([128, 4, 64]),
                    mul)
                if prev is not None:
                    seg_mm(*prev)
                if i == 6 and b + 1 < nblk:
                    z_next, e_next = prep_block(b + 1, xt_next, False)
                prev = (t, oh_cur, z_cur, i)
            seg_mm(*prev)

            nc.vector.tensor_copy(stage[:, 0], acc[0])
            nc.vector.tensor_copy(stage[:, 1], acc[1])
            nc.sync.dma_start(
                part_d[:], stage.rearrange("p a b -> p (a b)"))

    nc.compile()
    return nc


def _get_prog(nblk):
    if nblk not in _prog_cache:
        _prog_cache[nblk] = _build(nblk)
    return _prog_cache[nblk]


def _host_weights(lin_w, lin_b, att_w, att_b):
    # wvh col layout [head, 64]: wvh[k, h*64+j] = lin_w[h*64+j, k]
    wvh = np.ascontiguousarray(lin_w.T).astype(np.float16)        # [128, 256]
    w3 = lin_w.reshape(NHEAD, OUT_CH, IN_CH).astype(np.float64)
    v = np.einsum("hjk,j->kh", w3, att_w[0].astype(np.float64))   # [128, 4]
    wvv = v.astype(np.float16)
    c = (lin_b.reshape(NHEAD, OUT_CH).astype(np.float64)
         @ att_w[0].astype(np.float64) + float(att_b[0]))          # [4]
    cvec = np.tile(np.tile(c.astype(np.float16), 8), (128, 1))  # [128, 32]
    return {"wvh": wvh, "wvv": wvv, "cvec": cvec}


def _prepare(x, y, lin_w, lin_b, att_w, att_b):
    """Sort rows by class, shard into disjoint 125-class windows, build
    per-core input maps.  Returns (nblk, in_maps)."""
    order = np.argsort(y, kind="stable")
    ys = y[order]
    bounds = np.searchsorted(
        ys, np.arange(0, NUM_CLASSES + 1, CLS_PER_CORE)).astype(np.int64)
    max_rows = int(np.diff(bounds).max())
    nblk = max(NBLK, -(-max_rows // ROWS_PER_BLOCK))
    nrows = nblk * ROWS_PER_BLOCK
    slot_eye = np.eye(128, dtype=np.float16)
    wmap = _host_weights(lin_w, lin_b, att_w, att_b)
    in_maps = []
    for k in range(NCORES):
        lo, hi = int(bounds[k]), int(bounds[k + 1])
        r = hi - lo
        xt = np.zeros((128, nrows), dtype=np.float16)
        xt[:, :r] = x[order[lo:hi]].T.astype(np.float16)
        spad = np.full(nrows, DUMP_SLOT, dtype=np.int32)
        spad[:r] = ys[lo:hi] - k * CLS_PER_CORE
        oh = slot_eye[spad]                                # [nrows, 128]
        m = {"xt": xt, "oh": oh}
        m.update(wmap)
        in_maps.append(m)
    return nblk, in_maps


def kernel(context_h_input, context_y, num_classes, lin_w, lin_b, att_w,
           att_b):
    from concourse.bass_utils import run_bass_kernel_spmd

    x = np.asarray(context_h_input, dtype=np.float32)
    y = np.asarray(context_y, dtype=np.int32)
    lin_w = np.asarray(lin_w, dtype=np.float32)
    lin_b = np.asarray(lin_b, dtype=np.float32)
    att_w = np.asarray(att_w, dtype=np.float32)
    att_b = np.asarray(att_b, dtype=np.float32)
    n = x.shape[0]
    assert int(num_classes) == NUM_CLASSES and n == N_TOTAL

    nblk, in_maps = _prepare(x, y, lin_w, lin_b, att_w, att_b)
    nc = _get_prog(nblk)
    res = run_bass_kernel_spmd(nc, in_maps, list(range(NCORES)))

    pooled = np.empty((NUM_CLASSES, NHEAD, OUT_CH), dtype=np.float64)
    denom = np.empty((NUM_CLASSES, NHEAD), dtype=np.float64)
    for k, r in enumerate(res.results):
        p = r["part"].astype(np.float64).reshape(128, 2, 260).sum(axis=1)
        rc = p[:CLS_PER_CORE].reshape(CLS_PER_CORE, NHEAD, 65)
        pooled[k * CLS_PER_CORE:(k + 1) * CLS_PER_CORE] = rc[:, :, 0:64]
        denom[k * CLS_PER_CORE:(k + 1) * CLS_PER_CORE] = rc[:, :, 64]
    out = pooled / denom[:, :, None] + lin_b.astype(np.float64).reshape(
        NHEAD, OUT_CH)[None]
    return out.reshape(NUM_CLASSES, NHEAD * OUT_CH).astype(np.float32)
